# revision 26
# baseline (speedup 1.0000x reference)
"""BitNetLinear Trainium2 kernel (8 NeuronCores, SPMD data-parallel).

y = round(clip(x, +-127*s)/s)*s @ (ternary(W))^T + ternary(b)
with s = exp2(floor(log2(max|x|/127 + eps))) a power of two (global over x).

Sharding: batch dim (8) -> one batch of [4096, 1024] per core.

v2 design (vs the 208us v1):
 * TRANSPOSED GEMM: compute y^T with out_features on PSUM partitions
   (lhsT = W^T block stationary, quantized-x rows streaming). The ternary
   bias becomes per-partition and fuses into the ACT-engine PSUM->SBUF
   copy (activation Identity: out = ps*c + b) - no DVE bias pass, stores
   depend only on PE+ACT. Host transposes y back (layout only).
 * HARDCODED SPECULATIVE SCALE: for this input regime (randn) the global
   power-of-two scale is 2^-5 with overwhelming probability
   (P(other binade) < 1e-7 for any randn(0,1) of this size). The kernel
   runs the whole pipeline with s_spec = 2^-5 baked in, so the first
   matmul starts as soon as chunk 0 lands - no absmax on the critical
   path. Exactness is unconditional: per-chunk absmaxes accumulate off
   the critical path, a 512B AllReduce(max) produces the true global
   max, and a one-instruction binade check (sign(v-s) XOR sign(v-2s))
   branches to an exact full redo with the device-computed scale if the
   speculation missed (ANY input remains bit-correct, just slower).
 * EAGER x RESIDENCY: all 16 MiB of the x shard is loaded up front into
   SBUF (it fits), so loads never pace compute, the AllReduce finishes
   long before the last matmul, and the redo path (if taken) reads x
   from SBUF without reloading.

x is quantized to integer-valued bf16 (round-half-even via the +-1.5*2^23
trick); the bf16 matmul with fp32 PSUM accumulation is exact integer
arithmetic (|x_int| <= 127, w in {-1,0,1}, |acc| < 2^24); the result is
scaled by s*gamma_w and the ternary bias added, all in the ACT copy.
"""

import numpy as np
import ml_dtypes
from contextlib import ExitStack

import concourse.bass as bass
import concourse.mybir as mybir
import concourse.tile as tile
from concourse import bacc, bass_isa, bass_utils

F32 = mybir.dt.float32
BF16 = mybir.dt.bfloat16
I32 = mybir.dt.int32

N_CORES = 8
P = 128
IN_F = 1024
OUT_F = 1024
KC = IN_F // P          # 8 contraction chunks of 128
JC = OUT_F // P         # 8 output blocks of 128
RSUB = 512              # rows per chunk / row-group
ROUND_C = 12582912.0    # 1.5 * 2**23: float32 round-half-even trick
EPS = 1e-8
S_SPEC = 2.0 ** -5      # speculative global scale (binade of max|x|/127+eps)
INV_SPEC = 2.0 ** 5


def _emit_scale_chain(nc, consts, gmax, gamma_sb, mask_t, expc_t, tag):
    """From a [P,1] absmax tile, compute s = exp2(floor(log2(m/127+eps)))
    via exponent masking, 1/s via exponent arithmetic, and c = s*gamma."""
    v_t = consts.tile([P, 1], F32, tag=f"v_{tag}")
    nc.vector.tensor_scalar(
        out=v_t,
        in0=gmax,
        scalar1=float(np.float32(1.0 / 127.0)),
        scalar2=float(np.float32(EPS)),
        op0=mybir.AluOpType.mult,
        op1=mybir.AluOpType.add,
    )
    s_t = consts.tile([P, 1], F32, tag=f"s_{tag}")
    nc.vector.tensor_tensor(
        out=s_t.bitcast(I32),
        in0=v_t.bitcast(I32),
        in1=mask_t,
        op=mybir.AluOpType.bitwise_and,
    )
    inv_t = consts.tile([P, 1], F32, tag=f"inv_{tag}")
    nc.vector.tensor_tensor(
        out=inv_t.bitcast(I32),
        in0=expc_t,
        in1=s_t.bitcast(I32),
        op=mybir.AluOpType.subtract,
    )
    c_t = consts.tile([P, 1], F32, tag=f"c_{tag}")
    nc.vector.tensor_mul(out=c_t, in0=s_t, in1=gamma_sb)
    return s_t, inv_t, c_t


def _emit_phase(nc, pools, nt, xc_tiles, yT, w_sb, bias_sb, inv, c_scale,
                rg_hook=None, store_engine=None, split_rg0=False):
    """Quantize x with 1/s (DVE mult+max, min+addC; ACT subC->bf16), then
    per row-group run the transposed matmul (W^T blocks stationary, xi rows
    streaming, PSUM partition dim = out_features), fuse scale+bias into the
    ACT PSUM->SBUF copy, store y^T tiles. `inv` is a float or [P,1] tile;
    `c_scale` a [P,1] tile. Returns last emitted instruction per engine.

    Stores go out on `store_engine` (default nc.gpsimd). The sync/HWDGE
    ring is preferred for the fast phase: SWDGE costs ~2us of Q7
    descriptor generation per dma, which backs up 64 stores and stalls
    the ACT copies on yo-pool recycling.

    rg_hook(rg, xc) is called one row-group LATE (after rg+1's quantize
    ops) so its absmax reduce never delays the next group's xi production.

    split_rg0 runs row-group 0's matmuls as two k-sweeps (k 0..3 then
    4..7, PSUM accumulation group held open across the sweeps) so the
    first 32 matmuls depend only on the first halves of w_sb and chunk 0.

    Quantize chain (exact): clip-before-round equals round-then-clip since
    the bounds are integers, so
      t1 = max(x*inv, -127); t1 = min(t1, 127) + C; xi = bf16(t1 - C)
    yields round-half-even(clip(x/s)) exactly (+-1.5*2^23 trick; mult by a
    power of two is exact; integer results |.|<=127 are bf16-exact)."""
    t1_pool, xi_pool, yo_pool, ps_pool = pools
    if store_engine is None:
        store_engine = nc.gpsimd
    last = {}

    def emit_quant(rg, pieces=2):
        """Quantize chunk rg in `pieces` equal k-ranges (finer pieces for
        chunk 0 let the first matmuls start before the whole chunk lands)."""
        xc = xc_tiles[rg]
        xi_slices = []
        kw = KC // pieces
        for h in range(pieces):
            t1 = t1_pool.tile([P, kw, RSUB], F32, tag="t1")
            nc.vector.tensor_scalar(
                out=t1,
                in0=xc[:, h * kw: (h + 1) * kw, :],
                scalar1=inv,
                scalar2=-127.0,
                op0=mybir.AluOpType.mult,
                op1=mybir.AluOpType.max,
            )
            last["DVE"] = nc.vector.tensor_scalar(
                out=t1,
                in0=t1,
                scalar1=127.0,
                scalar2=ROUND_C,
                op0=mybir.AluOpType.min,
                op1=mybir.AluOpType.add,
            )
            for kk in range(kw):
                xi = xi_pool.tile([P, RSUB], BF16, tag="xi")
                last["ACT"] = nc.scalar.activation(
                    out=xi,
                    in_=t1[:, kk, :],
                    func=mybir.ActivationFunctionType.Copy,
                    bias=-ROUND_C,
                    scale=1.0,
                )
                xi_slices.append(xi)
        return xi_slices

    yT_q = yT.rearrange("(a p) r -> p a r", p=P)
    QB = 4  # output tiles batched per store dma (SWDGE fixed cost ~2us)

    def emit_mm_out(rg, xi_slices, sweeps=None):
        """sweeps: list of k-ranges; the PSUM accumulation group of each
        bank is held open across the sweeps so earlier sweeps can run
        before later k-slices (chunk-0 pipelining)."""
        if sweeps is None:
            sweeps = [(0, KC)]
        ps_tiles = []
        for j in range(JC):
            ps = ps_pool.tile([P, RSUB], F32, tag="ps")
            ps_tiles.append(ps)
        multi = len(sweeps) > 1
        for ka, kb in sweeps:
            for j in range(JC):
                for k in range(ka, kb):
                    last["PE"] = nc.tensor.matmul(
                        ps_tiles[j],
                        lhsT=w_sb[:, k, j * P: (j + 1) * P],
                        rhs=xi_slices[k],
                        start=(k == 0),
                        stop=(k == KC - 1),
                        skip_group_check=multi,
                    )
        yo = None
        for j in range(JC):
            if j % QB == 0:
                yo = yo_pool.tile([P, QB, RSUB], F32, tag="yo")
            # scale+bias fused into the PSUM->SBUF copy, on DVE (two [P,1]
            # AP scalars) - keeps the ACT engine dedicated to the xi casts
            last["DVE"] = nc.vector.tensor_scalar(
                out=yo[:, j % QB, :],
                in0=ps_tiles[j],
                scalar1=c_scale,
                scalar2=bias_sb[:, j: j + 1],
                op0=mybir.AluOpType.mult,
                op1=mybir.AluOpType.add,
            )
            if j % QB == QB - 1:
                last["ST"] = store_engine.dma_start(
                    out=yT_q[:, j - QB + 1: j + 1,
                             rg * RSUB: (rg + 1) * RSUB],
                    in_=yo,
                )

    # Software pipeline with a one-group quantize lead: group rg's
    # quantize (DVE passes + ACT bf16 casts) is emitted BEFORE group
    # rg-1's matmul/copy/store section. The ACT engine's FIFO is strict;
    # without the lead, pass3(rg) sits behind copies(rg-1), which wait on
    # rg-1's matmuls - serializing xi production with PE progress.
    xi_prev = None
    for rg in range(nt):
        if split_rg0 and rg == 0:
            xi_now = emit_quant(0, pieces=4)
        else:
            xi_now = emit_quant(rg)
        if rg >= 1:
            # lagged absmax: after the NEXT group's quantize, off the
            # xi critical path
            if rg_hook is not None:
                rg_hook(rg - 1, xc_tiles[rg - 1])
            emit_mm_out(rg - 1, xi_prev,
                        sweeps=[(0, 2), (2, 4), (4, 8)]
                        if (split_rg0 and rg == 1) else None)
        xi_prev = xi_now
    if rg_hook is not None:
        rg_hook(nt - 1, xc_tiles[nt - 1])
    emit_mm_out(nt - 1, xi_prev)
    return last


def build_program(rows: int = 4096, num_cores: int = N_CORES,
                  safe: bool = False) -> bacc.Bacc:
    """safe=False: fast program - speculative scale hardcoded, binade-check
    verdict exported to DRAM (host re-runs the safe program on mispredict).
    safe=True: self-contained exact program with an on-device If/redo."""
    assert rows % RSUB == 0
    nc = bacc.Bacc(
        "TRN2",
        target_bir_lowering=False,
        debug=False,
        enable_asserts=False,
        num_devices=num_cores,
    )
    nt = rows // RSUB
    # x shard pre-tiled on host: xt[t, p, c, r] = x[t*RSUB + r, c*P + p],
    # so every chunk load is fully contiguous.
    xt = nc.dram_tensor("xt", (nt, P, KC, RSUB), F32, kind="ExternalInput").ap()
    wq = nc.dram_tensor("wq", (IN_F, OUT_F), BF16, kind="ExternalInput").ap()
    # bias pre-transposed on host to [P, JC]: bqt[p, j] = bq[j*128 + p]
    bqt = nc.dram_tensor("bqt", (P, JC), F32, kind="ExternalInput").ap()
    gq = nc.dram_tensor("gq", (1, 1), F32, kind="ExternalInput").ap()
    # transposed output y^T [out_features, rows]; host transposes back
    yT = nc.dram_tensor("yT", (OUT_F, rows), F32, kind="ExternalOutput").ap()
    # binade-check verdict (int32 bits; >= 0 means speculation missed)
    chk_out = nc.dram_tensor("chk", (P, 1), I32, kind="ExternalOutput").ap()
    # Collectives cannot target I/O tensors; bounce through internal DRAM.
    cc_in = nc.dram_tensor("cc_in", (P, 1), F32).ap()
    cc_out = nc.dram_tensor("cc_out", (P, 1), F32).ap()
    # dummy collective to pre-warm the ncfw/collectives firmware
    ccw_in = nc.dram_tensor("ccw_in", (P, 1), F32).ap()
    ccw_out = nc.dram_tensor("ccw_out", (P, 1), F32).ap()

    with tile.TileContext(nc, num_cores=num_cores) as tc, ExitStack() as ctx:
        consts = ctx.enter_context(tc.tile_pool(name="consts", bufs=1))

        mask_t = consts.tile([P, 1], I32)
        nc.vector.memset(mask_t, -8388608)  # 0xFF800000: sign+exponent mask
        expc_t = consts.tile([P, 1], I32)
        nc.vector.memset(expc_t, 0x7F000000)  # bits of (254<<23)

        # All big loads ride the sync ring (its HWDGE queue shards across
        # all 16 SDMA engines; the scalar ring's does not). Interleave the
        # weight halves with chunk 0's halves so the first matmul's inputs
        # land together as early as possible.
        gamma_sb = consts.tile([P, 1], F32)
        w_sb = consts.tile([P, KC, OUT_F], BF16)
        bias_sb = consts.tile([P, JC], F32)
        w_src = wq.rearrange("(c p) o -> p c o", p=P)
        c_spec = consts.tile([P, 1], F32)
        partials = consts.tile([P, nt], F32)
        gmax_g = consts.tile([P, 1], F32)
        warm_rhs = consts.tile([P, RSUB], BF16)
        nc.vector.memset(warm_rhs, 0.0)
        warm_f = consts.tile([P, 1], F32)
        nc.vector.memset(warm_f, 1.0)
        warm_o = consts.tile([P, 1], F32)
        rg_cc = [list(range(num_cores))]

        with (
            tc.tile_pool(name="xc", bufs=nt) as xc_pool,
            tc.tile_pool(name="t1", bufs=2) as t1_pool,
            tc.tile_pool(name="xi", bufs=3 * KC) as xi_pool,
            tc.tile_pool(name="yo", bufs=2) as yo_pool,
            tc.tile_pool(name="ps", bufs=8, space="PSUM") as ps_pool,
        ):
            pools = (t1_pool, xi_pool, yo_pool, ps_pool)

            # eager x loads: all chunks issued up front on the sync ring,
            # two half-chunk DMAs per chunk (finer landing granularity).
            # Issue order front-loads what the first matmul needs:
            # w half 0, chunk0 half 0, w half 1, chunk0 half 1, consts,
            # then chunks 1..nt-1.
            xc_tiles = []
            for _t in range(nt):
                xc = xc_pool.tile([P, KC, RSUB], F32, tag="xc")
                xc_tiles.append(xc)

            def load_x(t, ka, kb):
                return nc.sync.dma_start(
                    out=xc_tiles[t][:, ka:kb, :],
                    in_=xt[t, :, ka:kb, :],
                )

            def load_w(ka, kb):
                return nc.sync.dma_start(out=w_sb[:, ka:kb, :],
                                         in_=w_src[:, ka:kb, :])

            # quarter-granular interleave for what the first matmuls need;
            # later chunks as single 2 MiB dmas (fewer in-flight slots on
            # the ring's completion-semaphore rotation)
            nc.sync.dma_start(out=gamma_sb, in_=gq.to_broadcast((P, 1)))
            load_w(0, 2)
            load_x(0, 0, 2)
            load_w(2, 4)
            load_x(0, 2, 4)
            load_w(4, KC)
            load_x(0, 4, KC)
            load_x(1, 0, KC // 2)
            load_x(1, KC // 2, KC)
            nc.sync.dma_start(out=bias_sb, in_=bqt)
            load_x(2, 0, KC // 2)
            load_x(2, KC // 2, KC)
            last_sp = None
            for t in range(3, nt):
                last_sp = load_x(t, 0, KC)

            # c = s_spec * gamma for the speculative fast path
            nc.vector.tensor_scalar(
                out=c_spec, in0=gamma_sb, scalar1=S_SPEC, scalar2=None,
                op0=mybir.AluOpType.mult,
            )

            # --- warmups, all dependency-free ---
            # PE: junk matmuls flip HAM to full clock before the first
            # real matmul
            warm_ps = ps_pool.tile([P, RSUB], F32, tag="ps")
            for _ in range(12):
                nc.tensor.matmul(
                    warm_ps, lhsT=warm_rhs[:, 0:P], rhs=warm_rhs,
                    start=True, stop=True,
                )
            # gpsimd: pre-load the Q7 reduce library with a dummy reduce
            nc.gpsimd.tensor_reduce(
                out=warm_o[0:1, 0:1], in_=warm_f,
                axis=mybir.AxisListType.XYZWC,
                op=mybir.AluOpType.max, apply_absolute_value=True,
            )
            # collectives firmware: dummy 512B AllReduce so the real one
            # later starts without the ncfw cold-wake latency
            nc.gpsimd.dma_start(out=ccw_in, in_=warm_f)
            nc.gpsimd.collective_compute(
                "AllReduce", mybir.AluOpType.max, replica_groups=rg_cc,
                ins=[ccw_in.opt()], outs=[ccw_out.opt()],
            )

            def rg_hook(rg, xc):
                # per-chunk absmax on GPSIMD (whole chunk -> [1,1]): off
                # both the DVE xi path and the ACT path entirely; the x
                # chunk is SBUF-resident so it can run any time after its
                # load without blocking stores queued behind it
                nc.gpsimd.tensor_reduce(
                    out=partials[0:1, rg: rg + 1],
                    in_=xc,
                    axis=mybir.AxisListType.XYZWC,
                    op=mybir.AluOpType.max,
                    apply_absolute_value=True,
                )
                if rg == nt - 1:
                    # local max scalar -> 4B AllReduce(max) across the 8
                    # cores; completes well before the last matmul
                    gmax_sc = consts.tile([1, 1], F32)
                    nc.gpsimd.tensor_reduce(
                        out=gmax_sc, in_=partials[0:1, :],
                        axis=mybir.AxisListType.XYZWC,
                        op=mybir.AluOpType.max,
                    )
                    nc.gpsimd.dma_start(out=cc_in[0:1, 0:1], in_=gmax_sc)
                    nc.gpsimd.collective_compute(
                        "AllReduce", mybir.AluOpType.max, replica_groups=rg_cc,
                        ins=[cc_in.opt()], outs=[cc_out.opt()],
                    )
                    # broadcast the global max to all partitions (the redo
                    # path's scale chain needs a [P,1] operand)
                    nc.gpsimd.dma_start(
                        out=gmax_g,
                        in_=cc_out[0:1, 0:1].to_broadcast((P, 1)),
                    )

            last = _emit_phase(
                nc, pools, nt, xc_tiles, yT, w_sb, bias_sb,
                INV_SPEC, c_spec, rg_hook=rg_hook,
                split_rg0=True,
            )
            last["SP"] = last_sp
            last["POOL"] = last.get("ST")

            # --- speculation check: s_spec is the true scale iff
            # v = gmax/127+eps lies in [s_spec, 2*s_spec), i.e.
            # sign(v - s) != sign(v - 2s). XOR of the float bits makes the
            # pass condition a single sign test: chk < 0 <=> in-binade. ---
            v_g = consts.tile([P, 1], F32)
            nc.vector.tensor_scalar(
                out=v_g,
                in0=gmax_g,
                scalar1=float(np.float32(1.0 / 127.0)),
                scalar2=float(np.float32(EPS)),
                op0=mybir.AluOpType.mult,
                op1=mybir.AluOpType.add,
            )
            a_t = consts.tile([P, 1], F32)
            nc.vector.tensor_scalar(
                out=a_t, in0=v_g, scalar1=-S_SPEC, scalar2=None,
                op0=mybir.AluOpType.add,
            )
            b_t = consts.tile([P, 1], F32)
            nc.vector.tensor_scalar(
                out=b_t, in0=v_g, scalar1=-2.0 * S_SPEC, scalar2=None,
                op0=mybir.AluOpType.add,
            )
            chk = consts.tile([P, 1], I32)
            last["DVE"] = nc.vector.tensor_tensor(
                out=chk, in0=a_t.bitcast(I32), in1=b_t.bitcast(I32),
                op=mybir.AluOpType.bitwise_xor,
            )
            if not safe:
                # fast program: export the verdict; the host re-runs the
                # safe program in the (P < 1e-7) mispredict case. No
                # on-device branch means no skipped-region semaphore
                # reconciliation in the epilogue.
                nc.gpsimd.dma_start(out=chk_out, in_=chk)
            else:
                last["POOL"] = nc.gpsimd.dma_start(out=chk_out, in_=chk)
                regs = nc.alloc_registers(
                    "spec_chk",
                    bass.OrderedSet([
                        mybir.EngineType.SP,
                        mybir.EngineType.DVE,
                        mybir.EngineType.Activation,
                        mybir.EngineType.PE,
                        mybir.EngineType.Pool,
                    ]),
                )
                # Pin each engine's reg_load after its last speculative-
                # phase instruction: the load waits on the AllReduce, and
                # the Tile scheduler would otherwise be free to place it
                # mid-stream, stalling that engine's FIFO on the collective.
                eng_key = {
                    mybir.EngineType.PE: "PE",
                    mybir.EngineType.DVE: "DVE",
                    mybir.EngineType.Activation: "ACT",
                    mybir.EngineType.SP: "SP",
                    mybir.EngineType.Pool: "POOL",
                }
                for reg in regs:
                    ld = nc.reg_load(reg, chk[0:1, 0:1])
                    prev = last.get(eng_key[reg.engine])
                    if prev is not None:
                        tile.add_dep_helper(
                            ld.ins, prev.ins, sync=False,
                            reason="speculation check after spec phase",
                        )
                # negative iff inside the speculated binade (fast path)
                with tc.If(nc.snap(regs) >= 0):
                    # mismatch: redo with the exact global scale
                    # (x chunks are still resident in SBUF - no reloads)
                    _, inv_g, c_g = _emit_scale_chain(
                        nc, consts, gmax_g, gamma_sb, mask_t, expc_t, "g")
                    _emit_phase(nc, pools, nt, xc_tiles, yT, w_sb, bias_sb,
                                inv_g, c_g)

    nc.compile()
    return nc


def quantize_params(weight: np.ndarray, bias: np.ndarray):
    """Ternary-quantize weight/bias exactly as the reference (f64 math whose
    f32 rounding matches jax-f32; verified margins are orders of magnitude
    above f32 accumulation differences)."""
    w64 = weight.astype(np.float64)
    g_w = np.float32(np.abs(w64).mean())
    wi = np.clip(np.round(w64 / (np.float64(g_w) + EPS)), -1.0, 1.0)
    b64 = bias.astype(np.float64)
    g_b = np.float32(np.abs(b64).mean())
    bi = np.clip(np.round(b64 / (np.float64(g_b) + EPS)), -1.0, 1.0)
    bq = (bi * np.float64(g_b)).astype(np.float32)  # exact: {-g_b, 0, g_b}
    return wi, g_w, bq


_PROGRAM_CACHE: dict[tuple[int, bool], bacc.Bacc] = {}


def _get_program(rows: int, safe: bool = False) -> bacc.Bacc:
    key = (rows, safe)
    if key not in _PROGRAM_CACHE:
        _PROGRAM_CACHE[key] = build_program(rows, safe=safe)
    return _PROGRAM_CACHE[key]


def tile_x_shard(x2d: np.ndarray) -> np.ndarray:
    """[rows, IN_F] -> [nt, P, KC, RSUB] with xt[t,p,c,r] = x[t*RSUB+r, c*P+p]."""
    rows = x2d.shape[0]
    return np.ascontiguousarray(
        x2d.reshape(rows // RSUB, RSUB, KC, P).transpose(0, 3, 2, 1)
    )


def prepare_in_maps(x: np.ndarray, weight: np.ndarray, bias: np.ndarray):
    x = np.asarray(x, dtype=np.float32)
    weight = np.asarray(weight, dtype=np.float32)
    bias = np.asarray(bias, dtype=np.float32)
    batch, rows, in_f = x.shape
    assert batch == N_CORES and in_f == IN_F and weight.shape == (OUT_F, IN_F)

    wi, g_w, bq = quantize_params(weight, bias)
    wq_t = np.ascontiguousarray(wi.T).astype(ml_dtypes.bfloat16)  # [in, out]
    bqt = np.ascontiguousarray(bq.reshape(JC, P).T)               # [P, JC]
    gq = np.array([[g_w]], dtype=np.float32)

    in_maps = []
    for c in range(N_CORES):
        in_maps.append(
            {
                "xt": tile_x_shard(x[c]),
                "wq": wq_t,
                "bqt": bqt,
                "gq": gq,
            }
        )
    return in_maps, rows


def kernel(x: np.ndarray, weight: np.ndarray, bias: np.ndarray) -> np.ndarray:
    in_maps, rows = prepare_in_maps(x, weight, bias)
    nc = _get_program(rows)
    res = bass_utils.run_bass_kernel_spmd(nc, in_maps, core_ids=list(range(N_CORES)))
    # device-computed binade check: int32 bits of the XOR sign test are
    # negative iff the speculated scale is the true global scale
    if any(int(res.results[c]["chk"][0, 0]) >= 0 for c in range(N_CORES)):
        # speculation missed (P < 1e-7 for randn inputs): run the
        # self-contained exact program with the on-device redo branch
        nc_safe = _get_program(rows, safe=True)
        res = bass_utils.run_bass_kernel_spmd(
            nc_safe, in_maps, core_ids=list(range(N_CORES)))
    return np.stack(
        [np.ascontiguousarray(res.results[c]["yT"].T) for c in range(N_CORES)],
        axis=0,
    )
